# revision 23
# baseline (speedup 1.0000x reference)
"""Trainium2 Bass kernel for batched 22-node complete-digraph GNN.

Model (per reference):
    x0 = relu(features @ W_lift + b_lift)            # [N, 15]
    agg1 = segment_sum(x0[src], dst)                 # complete digraph w/ self
    x1 = relu(agg1 @ W1 + b1)
    agg2 = segment_sum(x1[src], dst)
    x2 = relu(agg2 @ W2 + b2)
    out = x2.reshape(B, 110) @ W_ro + b_ro           # [B, 1]

Each 22-node graph is a complete digraph with self-loops, so the edge
aggregation is "every node receives the sum over all 22 nodes of its graph":
    S1_g   = sum_{i in graph g} relu(lift(f_i))      # [B, 15]
    h1_g   = relu(W1^T S1_g + b1)                    # same for all nodes of g
    x2_g   = relu((22*h1_g) @ W2 + b2)               # [B, 5]
    out_g  = x2_g @ (W_ro.reshape(22,5).sum(0)) + b_ro

Sharding: data-parallel over graphs, B=32768 split across 8 cores
(4096 graphs / 90112 nodes per core).  No cross-core communication.

Per-core schedule:
  - nodes in 8 blocks; block a holds features on partitions [9a,9a+9)
    (72 partitions), 11264 node columns, fp8-e4m3.
  - lift: fp8 DoubleRow matmul (k-split 72 = 2x36, lhsT [36,2,120],
    rhs [36,2,cols]) -> PSUM [120, cols] f32 at 0.5 cycles/col.
  - evac (bias+relu, PSUM f32 -> SBUF fp16) alternating ScalarE/VectorE
    per chunk.
  - reduce fused into the W1 matmul: 22 accumulating strided matmuls
    per chunk with the block-diag W1 lhsT: h1pre[80,g] = sum_i W1^T
    x0[:, g, i].  The graph-sum never touches VectorE.
  - stage-2 per segment: h1 evac, W2 matmul, evac, Wro matmul, bias
    evac, DMA out; emitted behind the lift pipeline.
"""

import os

import numpy as np

B = 32768
G = 22
N = B * G
NCORES = 8
BC = B // NCORES          # 4096 graphs per core
NC_NODES = BC * G         # 90112 nodes per core
NBLK = 8                  # node blocks per core (partition blocks)
CHUNK_NODES = NC_NODES // NBLK   # 11264 cols = 512 cgs * 22
NCG = CHUNK_NODES // G           # 512 column-groups (each = 8 graphs)
FT_P = 9 * NBLK           # 72 feature rows (split 2x36 for DoubleRow)
LIFT_P = 15 * NBLK        # 120 partitions for x0

WPACK_BYTES = 272         # fp16 W1/W2/Wro + fp32 biases, bitcast region


def _env(name, default):
    return os.environ.get(name, default)


# compute chunks (cols, multiples of 22)
CHUNKS = [int(c) for c in _env(
    "KERNEL_CHUNKS",
    "176," + ",".join(["880"] * 12) + ",528").split(",")]
assert sum(CHUNKS) == CHUNK_NODES and all(c % G == 0 for c in CHUNKS)
NCHUNK = len(CHUNKS)
CG_OF = []          # cg range per chunk
_g = 0
for _c in CHUNKS:
    CG_OF.append((_g, _g + _c // G))
    _g += _c // G

# per-chunk evac engine: A(ScalarE) / D(VectorE)
EVAC = _env("KERNEL_EVAC", "ADADADADADADAD")
assert len(EVAC) == NCHUNK and set(EVAC) <= {"A", "D"}

# ft DMA grouping: chunk 0 rides in the head DMA; groups of chunks per DMA
FTGROUP = [int(x) for x in _env("KERNEL_FTGROUP", "3,3,3,3,1").split(",")]
assert sum(FTGROUP) == NCHUNK - 1

# stage-2 segments (cg boundaries, must align to chunk boundaries)
SEGS = [int(x) for x in _env("KERNEL_SEGS", "0,168,368,512").split(",")]
assert SEGS[0] == 0 and SEGS[-1] == NCG

# stage-2 evac engines per segment: 3 chars (h1, h2, out) from {A, D}
SEGENG = _env("KERNEL_SEGENG", "ADA;DAD;ADA").split(";")
assert len(SEGENG) == len(SEGS) - 1

DELAY = int(_env("KERNEL_D", "2"))       # chunks between lift and seg emission
DELAY_MM = int(_env("KERNEL_DMM", "2"))  # chunks between lift and reduce-mms
NWARM = int(_env("KERNEL_WARM", "3"))

LAST_RESULT = None


def _structured(src, dst):
    offsets = np.repeat(np.arange(B, dtype=np.int32) * G, G * G)
    ls = np.tile(np.repeat(np.arange(G, dtype=np.int32), G), B)
    ld = np.tile(np.tile(np.arange(G, dtype=np.int32), G), B)
    return np.array_equal(src, offsets + ls) and np.array_equal(dst, offsets + ld)


def _fallback_numpy(features, src, dst, W_lift, b_lift, W1, b1, W2, b2, W_ro, b_ro):
    x = np.maximum(features @ W_lift + b_lift, 0.0)
    agg = np.zeros((N, x.shape[1]), np.float32)
    np.add.at(agg, dst, x[src])
    x = np.maximum(agg @ W1 + b1, 0.0)
    agg = np.zeros((N, x.shape[1]), np.float32)
    np.add.at(agg, dst, x[src])
    x = np.maximum(agg @ W2 + b2, 0.0)
    x = x.reshape(B, G * 5)
    return (x @ W_ro + b_ro).astype(np.float32)


def _block_diag(W, nblk, dtype):
    fi, fo = W.shape
    out = np.zeros((fi * nblk, fo * nblk), dtype)
    for a in range(nblk):
        out[fi * a:fi * (a + 1), fo * a:fo * (a + 1)] = W
    return out


_cached = {}


def _build_kernel():
    import concourse.bacc as bacc
    import concourse.mybir as mybir
    from concourse.tile import TileContext

    f8 = mybir.dt.float8e4
    f16 = mybir.dt.float16
    f32 = mybir.dt.float32
    Relu = mybir.ActivationFunctionType.Relu
    Ident = mybir.ActivationFunctionType.Identity
    Add = mybir.AluOpType.add
    Max = mybir.AluOpType.max
    DR = mybir.MatmulPerfMode.DoubleRow

    nc = bacc.Bacc(trn_type="TRN2", enable_partition_id=False)

    # head: [120, 256 (wl8, j-stride 128) | 272 pack | 2*CHUNKS[0] (ft c0)]
    head_cols = 256 + WPACK_BYTES + 2 * CHUNKS[0]
    head_d = nc.dram_tensor("head", [LIFT_P, head_cols], f8,
                            kind="ExternalInput")
    ft_d = nc.dram_tensor("ft", [FT_P, 2 * (CHUNK_NODES - CHUNKS[0])], f8,
                          kind="ExternalInput")
    out_d = nc.dram_tensor("out", [NBLK, NCG], f32, kind="ExternalOutput")

    starts = [sum(CHUNKS[:i]) for i in range(NCHUNK + 1)]

    grp_of_chunk = {}
    grp_cols = []    # (dram byte-col offset, group col width)
    ci = 1
    off = 0
    for gi, n in enumerate(FTGROUP):
        w = sum(CHUNKS[ci:ci + n])
        for t in range(ci, ci + n):
            grp_of_chunk[t] = (gi, starts[t] - starts[ci])
        grp_cols.append((off, w))
        off += 2 * w
        ci += n

    with TileContext(nc) as tc:
        with (
            tc.tile_pool(name="consts", bufs=1) as consts,
            tc.tile_pool(name="ft", bufs=1) as ftp,
            tc.tile_pool(name="x0", bufs=1) as x0p,
            tc.tile_pool(name="s1", bufs=1) as s1p,
            tc.tile_pool(name="ps", bufs=int(_env("KERNEL_PSBUFS", "3")),
                         space="PSUM") as psp,
        ):
            # ---- DMAs: head (weights + chunk 0) first, then ft groups
            head_sb = consts.tile([LIFT_P, head_cols], f8)
            nc.sync.dma_start(out=head_sb, in_=head_d[:, :])
            ft_tiles = []
            for gi, (off, w) in enumerate(grp_cols):
                tl = ftp.tile([FT_P, 2 * w], f8, tag=f"ftg{gi}", name=f"ftg{gi}")
                nc.sync.dma_start(out=tl, in_=ft_d[:, off:off + 2 * w])
                ft_tiles.append(tl)

            def ft_ap(t):
                """[36, 2, w] fp8 k-tile-interleaved view for chunk t."""
                if t == 0:
                    sl = head_sb[0:FT_P, 256 + WPACK_BYTES:
                                 256 + WPACK_BYTES + 2 * CHUNKS[0]]
                    return sl.rearrange("p (j n) -> p j n", j=2)
                gi, off = grp_of_chunk[t]
                t3 = ft_tiles[gi].rearrange("p (j n) -> p j n", j=2)
                return t3[:, :, off:off + CHUNKS[t]]

            wl8 = head_sb[0:FT_P, 0:256].rearrange(
                "p (j m) -> p j m", j=2)[:, :, 0:120]
            w1_sb = head_sb[:, 256:416].bitcast(f16)          # [120, 80]
            w2_sb = head_sb[0:10 * NBLK, 416:496].bitcast(f16)  # [80, 40]
            wro_sb = head_sb[0:5 * NBLK, 496:512].bitcast(f16)  # [40, 8]
            bias_f32 = head_sb[:, 512:528].bitcast(f32)       # [120, 4]
            blift = bias_f32[:, 0:1]
            b1 = bias_f32[0:10 * NBLK, 1:2]
            b2 = bias_f32[0:5 * NBLK, 2:3]
            bro = bias_f32[0:NBLK, 3:4]

            # ---- PE warm-up: tiny matmuls so the p-state ramp starts early
            warm_sb = consts.tile([FT_P, LIFT_P + 16], f16)
            nc.vector.memset(warm_sb, 0.0)
            warm_ps = psp.tile([LIFT_P, 16], f32, tag="h1acc", bufs=1, name="warm")
            for _ in range(NWARM):
                nc.tensor.matmul(warm_ps[:, :], lhsT=warm_sb[:, 0:LIFT_P],
                                 rhs=warm_sb[:, LIFT_P:LIFT_P + 16],
                                 start=True, stop=True)

            # absorb the const-DMA wait once per consumer engine
            dummy = consts.tile([LIFT_P, 4], f32)
            nc.scalar.copy(out=dummy, in_=bias_f32[:, 0:4])
            dummy2 = consts.tile([LIFT_P, 1], f32)
            nc.vector.tensor_copy(out=dummy2, in_=bias_f32[:, 0:1])

            h1acc = psp.tile([10 * NBLK, NCG], f32, tag="h1acc", bufs=1,
                             name="h1acc")
            h1_sb = s1p.tile([10 * NBLK, NCG], f16, tag="h1sb", name="h1sb")
            o_sb = s1p.tile([NBLK, NCG], f32, tag="osb", name="osb")

            x0_tiles = {}

            def evac_one(eng, out, in_, bias):
                if eng == "A":
                    nc.scalar.activation(out=out, in_=in_, func=Relu,
                                         bias=bias, scale=1.0)
                else:
                    nc.vector.tensor_scalar(out=out, in0=in_, scalar1=bias,
                                            scalar2=0.0, op0=Add, op1=Max)

            def emit_lift(t):
                ft_sb = ft_ap(t)
                w = CHUNKS[t]
                nc.tensor.ldweights(weights=ft_sb[:, :, 0:min(w, 64)])
                ps = psp.tile([LIFT_P, 880], f32, tag="ps", name=f"ps_{t}")
                col = 0
                while col < w:
                    n = min(512, w - col)
                    nc.tensor.matmul(ps[:, col:col + n], lhsT=wl8,
                                     rhs=ft_sb[:, :, col:col + n],
                                     start=True, stop=True, perf_mode=DR)
                    col += n
                return ps

            def emit_evac(t, ps):
                w = CHUNKS[t]
                x0 = x0p.tile([LIFT_P, w], f16, tag=f"x0_{t}", name=f"x0_{t}")
                evac_one(EVAC[t], x0[:, :], ps[:, 0:w], blift)
                x0_tiles[t] = x0

            def emit_reduce_mm(t):
                # 22 accumulating strided matmuls: h1pre = sum_i W1^T x0[:,:,i]
                g0, g1 = CG_OF[t]
                x3 = x0_tiles[t].rearrange("p (g i) -> p g i", i=G)
                for i in range(G):
                    nc.tensor.matmul(h1acc[:, g0:g1], lhsT=w1_sb,
                                     rhs=x3[:, :, i],
                                     start=(i == 0), stop=(i == G - 1))

            def emit_seg(si):
                g0, g1 = SEGS[si], SEGS[si + 1]
                w = g1 - g0
                e1, e2, e3 = SEGENG[si]
                evac_one(e1, h1_sb[:, g0:g1], h1acc[:, g0:g1], b1)
                # h2 and o share one PSUM bank (disjoint partitions)
                seg_ps = psp.tile([72, 512], f32, tag="s2",
                                  bufs=int(_env("KERNEL_S2BUFS", "1")),
                                  name=f"segps_{si}")
                h2_ps = seg_ps[0:5 * NBLK, :]
                o_ps = seg_ps[64:64 + NBLK, :]
                nc.tensor.matmul(h2_ps[:, 0:w], lhsT=w2_sb,
                                 rhs=h1_sb[:, g0:g1], start=True, stop=True)
                h2_sb = s1p.tile([5 * NBLK, 512], f16, tag=f"h2_{si}",
                                 name=f"h2sb_{si}")
                evac_one(e2, h2_sb[:, 0:w], h2_ps[:, 0:w], b2)
                nc.tensor.matmul(o_ps[:, 0:w], lhsT=wro_sb, rhs=h2_sb[:, 0:w],
                                 start=True, stop=True)
                if e3 == "A":
                    nc.scalar.activation(out=o_sb[:, g0:g1], in_=o_ps[:, 0:w],
                                         func=Ident, bias=bro, scale=1.0)
                else:
                    nc.vector.tensor_scalar(out=o_sb[:, g0:g1],
                                            in0=o_ps[:, 0:w], scalar1=bro,
                                            scalar2=None, op0=Add)
                nc.sync.dma_start(out=out_d[:, g0:g1], in_=o_sb[:, g0:g1])

            # backlog: reduce-mms flush before segments (kind 0 < 1)
            backlog = []   # (emit_at, kind, fn)
            for t in range(NCHUNK):
                backlog.append((t + DELAY_MM, 0,
                                lambda t=t: emit_reduce_mm(t)))
            for si in range(len(SEGS) - 1):
                ready = max(t for t in range(NCHUNK)
                            if CG_OF[t][1] <= SEGS[si + 1]
                            and CG_OF[t][0] >= SEGS[si])
                backlog.append((ready + DELAY, 1, lambda si=si: emit_seg(si)))
            backlog.sort(key=lambda x: (x[0], x[1]))

            bi = 0
            for t in range(NCHUNK):
                ps = emit_lift(t)
                emit_evac(t, ps)
                while bi < len(backlog) and backlog[bi][0] <= t:
                    backlog[bi][2]()
                    bi += 1
            for _, _, fn in sorted(backlog[bi:], key=lambda x: (x[1], x[0])):
                fn()

    if not nc.is_finalized():
        nc.finalize()
    return nc


def kernel(features, src, dst, W_lift, b_lift, W1, b1, W2, b2, W_ro, b_ro):
    global LAST_RESULT
    import ml_dtypes

    f8 = ml_dtypes.float8_e4m3fn

    features = np.asarray(features, np.float32)
    src = np.asarray(src, np.int32)
    dst = np.asarray(dst, np.int32)
    W_lift = np.asarray(W_lift, np.float32)
    b_lift = np.asarray(b_lift, np.float32)
    W1 = np.asarray(W1, np.float32)
    b1 = np.asarray(b1, np.float32)
    W2 = np.asarray(W2, np.float32)
    b2 = np.asarray(b2, np.float32)
    W_ro = np.asarray(W_ro, np.float32)
    b_ro = np.asarray(b_ro, np.float32)

    if not _structured(src, dst):
        return _fallback_numpy(features, src, dst, W_lift, b_lift,
                               W1, b1, W2, b2, W_ro, b_ro)

    # features -> per-core feature-major block layout [NCORES, 72, 11264]
    # fp8, then k-interleaved [NCORES, 36, 2, 11264]
    ft36 = (features.reshape(NCORES, NBLK, CHUNK_NODES, 9)
            .transpose(0, 1, 3, 2)
            .reshape(NCORES, FT_P, CHUNK_NODES)
            .astype(f8)
            .view(np.uint8)
            .reshape(NCORES, 2, 36, CHUNK_NODES)
            .transpose(0, 2, 1, 3))        # [NCORES, 36, 2, 11264]
    # duplicate the feature planes: rows 0:36 pair with W8, rows 36:72
    # pair with the quantization residual dW8
    ft = np.concatenate([ft36, ft36], axis=1)   # [NCORES, 72, 2, 11264]

    wlbd_f32 = _block_diag(W_lift, NBLK, np.float32)
    w8 = wlbd_f32.astype(f8)
    dw8 = (wlbd_f32 - w8.astype(np.float32)).astype(f8)
    w8u, dw8u = w8.view(np.uint8), dw8.view(np.uint8)
    wl8 = np.zeros((FT_P, 256), np.uint8)  # j-plane stride 128 (16B-aligned)
    wl8[0:36, 0:120] = w8u[0:36, :]
    wl8[0:36, 128:248] = w8u[36:72, :]
    wl8[36:72, 0:120] = dw8u[0:36, :]
    wl8[36:72, 128:248] = dw8u[36:72, :]

    wpack = np.zeros((LIFT_P, 136), np.float16)
    wpack[0:LIFT_P, 0:80] = _block_diag(W1, NBLK, np.float16)
    wpack[0:10 * NBLK, 80:120] = _block_diag((G * W2).astype(np.float32),
                                             NBLK, np.float16)
    wro_eff = W_ro.reshape(G, 5).sum(axis=0)
    for a in range(NBLK):
        wpack[5 * a:5 * (a + 1), 120 + a] = wro_eff

    bpack = np.zeros((LIFT_P, 4), np.float32)
    bpack[:, 0] = np.tile(b_lift, NBLK)
    bpack[0:10 * NBLK, 1] = np.tile(b1, NBLK)
    bpack[0:5 * NBLK, 2] = np.tile(b2, NBLK)
    bpack[0:NBLK, 3] = float(b_ro[0])
    wpack[:, 128:136] = bpack.view(np.float16)

    if "nc" not in _cached:
        _cached["nc"] = _build_kernel()
    nc = _cached["nc"]

    from concourse import bass_utils

    head_cols = 256 + WPACK_BYTES + 2 * CHUNKS[0]
    in_maps = []
    for c in range(NCORES):
        head = np.zeros((LIFT_P, head_cols), np.uint8)
        head[0:FT_P, 0:256] = wl8
        head[:, 256:528] = wpack.view(np.uint8)
        head[0:FT_P, 528:528 + 2 * CHUNKS[0]] = (
            ft[c, :, :, 0:CHUNKS[0]].reshape(FT_P, -1))
        slabs = []
        ci = 1
        for n in FTGROUP:
            c0, c1 = sum(CHUNKS[:ci]), sum(CHUNKS[:ci + n])
            slabs.append(np.ascontiguousarray(
                ft[c, :, :, c0:c1]).reshape(FT_P, -1))
            ci += n
        in_maps.append({
            "head": head.view(f8),
            "ft": np.concatenate(slabs, axis=1).view(f8),
        })

    trace = os.environ.get("KERNEL_TRACE", "0") == "1"
    res = None
    for attempt in range(4):
        try:
            res = bass_utils.run_bass_kernel_spmd(
                nc, in_maps, core_ids=list(range(NCORES)), trace=trace,
            )
            break
        except ModuleNotFoundError:
            trace = False
        except Exception as e:  # noqa: BLE001
            if attempt == 3 or "UNRECOVERABLE" not in str(e).upper():
                raise
            import time
            time.sleep(15)
    LAST_RESULT = res

    out = np.concatenate([r["out"].reshape(-1) for r in res.results])
    return np.ascontiguousarray(out.reshape(B, 1).astype(np.float32))
